# revision 38
# baseline (speedup 1.0000x reference)
"""Trainium2 Bass kernel for LocalRelationalLayer (sparse_attention).

Computation (per reference):
  xp = zero-pad(x, 3)                                   # [B,256,62,62]
  km = 1x1conv(xp, k_w)+k_b ; qm = 1x1conv(xp, q_w)+q_b # [B,32,.,.]
  E[b,cm,l,ky,kx] = exp(km[b,cm,r+ky,w+kx]*qm[b,cm,r+3,w+3] + gpk[cm,ky,kx])
  ck = E / sum_kx E                                     # softmax over kx only
  pre[b,m*32+cm,l] = sum_{ky,kx} ck * xp[b,m*32+cm,r+ky,w+kx]
  out = 1x1conv(pre, f_w)+f_b                           # [B,256,56,56]

Sharding: 8 cores = (b in 2) x (4 row-blocks of 14 output rows); halo rows in
the per-core slice; host concatenates. No collectives.

Per-core strategy ("all-packed"):
  - Attention weights are computed in a PACKED partition layout p = cm*4+g
    where g indexes ky within a group (group A: ky 0-3, group B: ky 4-6,
    slot g=3 duplicated/ignored).  This cuts the exp count 49 -> 14 and the
    softmax arithmetic 3.5x vs computing them 4x-replicated.
  - km is computed once on 32 partitions; cheap DMA remaps build the
    ky-shifted packed views (km4) and the per-head-group packed value views
    (xv_qm) -- one descriptor-friendly DMA each.
  - softmax denominators invert via the single-instruction DVE fast
    reciprocal (no ACT Ln/Exp table thrashing).
  - The value products PV = ck (x) xv happen as ONE fused DVE/Pool
    instruction per (group, head) over [128, 7kx, 784] using overlapping
    window access patterns.
  - All kx / ky-group / head-group summation is done by the otherwise-idle
    TensorEngine: selection matrices S (precomputed 0/1) matmul-accumulate
    every PV plane into the pre[] PSUM accumulators, also undoing the packed
    layout.  The final 1x1 conv then runs from SBUF copies of pre.
"""

import numpy as np
import ml_dtypes

B, C, H, W = 2, 256, 56, 56
K, PAD, M, CM = 7, 3, 8, 32
HP, WP = H + 2 * PAD, W + 2 * PAD      # 62, 62
RB = 4                                  # row blocks per batch
RH = H // RB                            # 14 output rows per core
RHP = RH + K                            # 21 stored rows per core (20 + 1 junk)
NCORES = 8
L = RH * W                              # 784 output positions per core

_bf16 = ml_dtypes.bfloat16
_PROGRAM = None

# column splits of the 784 positions into PSUM-bank-sized pieces
SPLITS = ((0, 392), (392, 392))


def _build_program():
    import concourse.bass as bass
    import concourse.tile as tile
    from concourse import bacc, mybir
    from concourse.ap import AP

    f32 = mybir.dt.float32
    bf16 = mybir.dt.bfloat16
    Exp = mybir.ActivationFunctionType.Exp
    Ident = mybir.ActivationFunctionType.Identity
    PS = bass.MemorySpace.PSUM

    nc = bacc.Bacc("TRN2", target_bir_lowering=False, debug=False,
                   num_devices=NCORES)

    xp_d = nc.dram_tensor("xp", [128, 2, RHP, WP], bf16, kind="ExternalInput")
    wk_d = nc.dram_tensor("wk", [128, 2, CM], bf16, kind="ExternalInput")
    wq_d = nc.dram_tensor("wq", [128, 2, 128], bf16, kind="ExternalInput")
    sm_d = nc.dram_tensor("sm", [128, 9, 128], bf16, kind="ExternalInput")
    fw_d = nc.dram_tensor("fw", [128, 4, 128], bf16, kind="ExternalInput")
    gpk_d = nc.dram_tensor("gpk", [128, 2 * K], f32, kind="ExternalInput")
    kb_d = nc.dram_tensor("kb", [CM, 1], f32, kind="ExternalInput")
    qb_d = nc.dram_tensor("qb", [128, 1], f32, kind="ExternalInput")
    fb_d = nc.dram_tensor("fb", [128, 2], f32, kind="ExternalInput")
    y_d = nc.dram_tensor("y", [128, 2, L], bf16, kind="ExternalOutput")

    NKM = RHP * WP          # 1302
    NQM = RH * WP           # 868

    def win_ap(t4):
        """[128, 7, 14, 56] overlapping kx-window view of a [128,14,62] tile."""
        a = t4[:]
        part = list(a.ap[0])
        return AP(tensor=a.tensor, offset=a.offset,
                  ap=[part, [1, K], [WP, RH], [1, W]])

    with tile.TileContext(nc) as tc:
        with (
            tc.tile_pool(name="inp", bufs=1) as inp,
            tc.tile_pool(name="wpool", bufs=1) as wpool,
            tc.tile_pool(name="kq", bufs=1) as kq,
            tc.tile_pool(name="att", bufs=1) as att,
            tc.tile_pool(name="pv", bufs=7) as pvp,
            tc.tile_pool(name="pvw", bufs=2) as pvw,
            tc.tile_pool(name="outp", bufs=1) as outp,
            tc.tile_pool(name="psMM", bufs=2, space=PS) as psMM,
            tc.tile_pool(name="psA", bufs=1, space=PS) as psA,
            tc.tile_pool(name="psB", bufs=2, space=PS) as psB,
        ):
            # ---- ACT table preload: absorb the 1283ns load off the path ----
            scratch = wpool.tile([32, 1], f32, tag="scr", name="scr")
            nc.gpsimd.memset(scratch[:], 0.0)
            nc.scalar.activation(scratch[:], scratch[:], Exp, bias=0.0,
                                 scale=1.0)

            # ---------------- input DMAs (SP queue, in need-order) -------
            xv = inp.tile([128, 2, RHP, WP], bf16, tag="xv", name="xv")
            nc.sync.dma_start(xv[:][:, 0], xp_d.ap()[:, 0])
            wk = wpool.tile([128, 2, CM], bf16, tag="wk", name="wk")
            nc.sync.dma_start(wk[:], wk_d.ap())
            kb = wpool.tile([CM, 1], f32, tag="kb", name="kb")
            nc.sync.dma_start(kb[:], kb_d.ap())
            nc.sync.dma_start(xv[:][:, 1], xp_d.ap()[:, 1])
            qb = wpool.tile([128, 1], f32, tag="qb", name="qb")
            nc.sync.dma_start(qb[:], qb_d.ap())
            wq = wpool.tile([128, 2, 128], bf16, tag="wq", name="wq")
            nc.sync.dma_start(wq[:], wq_d.ap())
            gpk = wpool.tile([128, 2 * K], f32, tag="gpk", name="gpk")
            nc.sync.dma_start(gpk[:], gpk_d.ap())
            smat = wpool.tile([128, 9, 128], bf16, tag="sm", name="sm")
            nc.sync.dma_start(smat[:], sm_d.ap())

            xq = [[None] * 4 for _ in range(4)]  # [grp*2+ci][mslot]

            def emit_xq(grp, ci, mslot):
                base = 0 if grp == 0 else 4
                t = kq.tile([128, RH, WP], bf16,
                            tag=f"xq{grp}{ci}{mslot}",
                            name=f"xq{grp}{ci}{mslot}")
                src0 = xv[:][mslot * 32:(mslot + 1) * 32, ci]
                part = list(src0.ap[0])
                src = AP(tensor=src0.tensor,
                         offset=src0.offset + base * WP,
                         ap=[part, [WP, 4], [WP, RH], [1, WP]])
                nc.sync.dma_start(t[:], src)
                xq[grp * 2 + ci][mslot] = t

            emit_xq(0, 0, 0)
            emit_xq(0, 0, 1)

            # km4 remaps are on the SP queue *early*: they gate P4/exp
            km32 = kq.tile([CM, RHP, WP], bf16, tag="km32", name="km32")
            km4 = []
            for grp in range(2):
                km4.append(kq.tile([128, RH, WP], bf16, tag=f"km4{grp}",
                                   name=f"km4{grp}"))

            def emit_km4(grp, h):
                # half h covers km4 rows 7h..7h+6  (km32 rows base+g+7h..)
                base = (0 if grp == 0 else 4) + 7 * h
                a = km32[:]
                part = list(a.ap[0])
                src = AP(tensor=a.tensor, offset=a.offset + base * WP,
                         ap=[part, [WP, 4], [WP, K], [1, WP]])
                nc.sync.dma_start(km4[grp][:][:, 7 * h:7 * h + K, :], src)

            # ---------------- km32 / qm4 matmuls ----------------
            km32_f = km32[:].rearrange("p r w -> p (r w)")
            for si, off in enumerate(range(0, NKM, 512)):
                n = min(512, NKM - off)
                ps = psMM.tile([128, 512], f32, tag="mm", name=f"psk{off}")
                for ci in range(2):
                    rhs = xv[:].rearrange("p c r w -> p (c r w)")[
                        :, ci * NKM + off: ci * NKM + off + n]
                    nc.tensor.matmul(ps[:CM, :n], wk[:, ci], rhs,
                                     start=(ci == 0), stop=(ci == 1))
                if si == 0:
                    nc.scalar.activation(km32_f[:, off:off + n], ps[:CM, :n],
                                         Ident, bias=kb[:], scale=1.0)
                elif si == 1:
                    nc.vector.tensor_scalar_add(km32_f[:, off:off + n],
                                                ps[:CM, :n], kb[:])
                else:
                    nc.vector.tensor_scalar_add(km32_f[:, off:off + n],
                                                ps[:CM, :n], kb[:])
            emit_km4(0, 0)
            emit_km4(0, 1)
            emit_km4(1, 0)
            emit_km4(1, 1)
            qm4 = kq.tile([128, RH, WP], bf16, tag="qm4", name="qm4")
            qm4_f = qm4[:].rearrange("p r w -> p (r w)")
            for off in range(0, NQM, 512):
                n = min(512, NQM - off)
                ps = psMM.tile([128, 512], f32, tag="mm", name=f"psq{off}")
                for ci in range(2):
                    rhs = xv[:].rearrange("p c r w -> p (c r w)")[
                        :, ci * NKM + PAD * WP + off: ci * NKM + PAD * WP + off + n]
                    nc.tensor.matmul(ps[:, :n], wq[:, ci], rhs,
                                     start=(ci == 0), stop=(ci == 1))
                if off == 0:
                    nc.scalar.activation(qm4_f[:, off:off + n], ps[:, :n],
                                         Ident, bias=qb[:], scale=1.0)
                else:
                    nc.vector.tensor_scalar_add(qm4_f[:, off:off + n],
                                                ps[:, :n], qb[:])

            # remaining value-view remaps + late weights on SP
            emit_xq(0, 0, 2)
            emit_xq(0, 0, 3)
            for mslot in range(4):
                emit_xq(0, 1, mslot)
            for ci in range(2):
                for mslot in range(4):
                    emit_xq(1, ci, mslot)
            fw = wpool.tile([128, 4, 128], bf16, tag="fw", name="fw")
            nc.sync.dma_start(fw[:], fw_d.ap())
            fb = wpool.tile([128, 2], f32, tag="fb", name="fb")
            nc.sync.dma_start(fb[:], fb_d.ap())

            qmc = qm4[:][:, :, PAD:PAD + W]  # [128, 14, 56] center query
            ident = smat[:, 8]               # [128, 128] identity

            def bcast_kx(ap2d):
                return ap2d.unsqueeze(1).broadcast_to((128, K, ap2d.shape[1]))

            # ---------------- attention (packed) ----------------
            # per-kx P4 muls; exps/d/recip/ck all split into position halves
            # (rows 0:7 / 7:14) so the normalized weights stream out early.
            P4 = [att.tile([128, K, RH, W], bf16, tag=f"P4{g}", name=f"P4{g}")
                  for g in range(2)]
            E4 = [att.tile([128, K, L], bf16, tag=f"E4{g}", name=f"E4{g}")
                  for g in range(2)]
            dps = [[psB.tile([128, n], f32, tag="pso", name=f"d{g}{si}")
                    for si, (o, n) in enumerate(SPLITS)] for g in range(2)]
            rf = [att.tile([128, L], f32, tag=f"rf{g}", name=f"rf{g}")
                  for g in range(2)]
            rb = [att.tile([128, L], bf16, tag=f"rb{g}", name=f"rb{g}")
                  for g in range(2)]
            ck4 = [att.tile([128, K, L], bf16, tag=f"ck{g}", name=f"ck{g}")
                   for g in range(2)]

            def emit_attention(grp):
                nsplit = 3 if grp == 0 else 2
                for h in range(2):
                    for kx in range(K):
                        eng = nc.vector if kx < nsplit else nc.gpsimd
                        eng.tensor_mul(
                            P4[grp][:, kx, 7 * h:7 * h + K, :],
                            km4[grp][:][:, 7 * h:7 * h + K, kx:kx + W],
                            qmc[:, 7 * h:7 * h + K, :])
                for h, (o, n) in enumerate(SPLITS):
                    for kx in range(K):
                        nc.scalar.activation(
                            E4[grp][:, kx, o:o + n],
                            P4[grp][:, kx].rearrange(
                                "p r w -> p (r w)")[:, o:o + n], Exp,
                            bias=gpk[:, grp * K + kx:grp * K + kx + 1],
                            scale=1.0)

            df_sb = att.tile([128, L], f32, tag="df0", name="df0")

            def emit_dsum(grp):
                for si, (o, n) in enumerate(SPLITS):
                    for kx in range(K):
                        nc.tensor.matmul(dps[grp][si][:], ident,
                                         E4[grp][:, kx, o:o + n],
                                         start=(kx == 0), stop=(kx == K - 1))

            def emit_dsum_pool(grp):
                # pairwise tree on the (idle) Pool; df_sb in f32 for the recip
                for h, (o, n) in enumerate(SPLITS):
                    E = E4[grp][:]
                    a = att.tile([128, 2, 392], bf16, tag=f"da{h}", name=f"da{grp}{h}")
                    nc.gpsimd.tensor_add(a[:], E[:, 0:2, o:o + n], E[:, 2:4, o:o + n])
                    b = att.tile([128, 392], bf16, tag=f"db{h}", name=f"db{grp}{h}")
                    nc.gpsimd.tensor_add(b[:], E[:, 4, o:o + n], E[:, 5, o:o + n])
                    c = att.tile([128, 392], bf16, tag=f"dc{h}", name=f"dc{grp}{h}")
                    nc.gpsimd.tensor_add(c[:], a[:, 0], a[:, 1])
                    d = att.tile([128, 392], bf16, tag=f"dd{h}", name=f"dd{grp}{h}")
                    nc.gpsimd.tensor_add(d[:], c[:], b[:])
                    nc.gpsimd.tensor_add(df_sb[:][:, o:o + n], d[:],
                                         E[:, 6, o:o + n])

            def emit_norm(grp, h):
                from concourse.dve_ops import (RECIPROCAL_APPROX_FAST,
                                               RECIP_APPROX_FAST_CONSTS)
                o, n = SPLITS[h]
                dsrc = dps[grp][h][:]
                nc.vector._custom_dve(RECIPROCAL_APPROX_FAST,
                                      out=rb[grp][:][:, o:o + n], in0=dsrc,
                                      **RECIP_APPROX_FAST_CONSTS)
                nc.vector.tensor_mul(
                    ck4[grp][:][:, :, o:o + n], E4[grp][:][:, :, o:o + n],
                    rb[grp][:][:, o:o + n].unsqueeze(1).broadcast_to(
                        (128, K, n)))

            emit_attention(0)
            emit_dsum(0)
            emit_attention(1)
            emit_norm(0, 0)
            emit_dsum(1)
            emit_norm(0, 1)

            # ---------------- value phase ----------------
            pre_ps = [[psA.tile([128, n], f32, tag=f"pre{ci}{si}",
                                name=f"pre{ci}{si}")
                       for si, (o, n) in enumerate(SPLITS)] for ci in range(2)]
            first = [[True] * 2 for _ in range(2)]
            UNITS = [(grp, ci, mslot) for grp in range(2)
                     for ci in range(2) for mslot in range(4)]
            # (ui, h) pairs: all grp-A half-0, then grp-A half-1, then grp-B
            PAIRS = ([(ui, 0) for ui in range(8)] + [(ui, 1) for ui in range(8)]
                     + [(ui, 0) for ui in range(8, 16)]
                     + [(ui, 1) for ui in range(8, 16)])
            last_pair = {}
            for pi, (ui, h) in enumerate(PAIRS):
                last_pair[(UNITS[ui][1], h)] = pi
            POOL_PAIRS = set(range(0, 32, 2))  # alternate Pool/DVE halves
            KNOB_PAIRS = {24, 26, 28, 30}      # Pool-produced, DVE presums
            for pi, (ui, h) in enumerate(PAIRS):
                grp, ci, mslot = UNITS[ui]
                o, n = SPLITS[h]
                PV = pvp.tile([128, K, 392], bf16, tag="PV", name=f"PV{pi}")
                eng = nc.gpsimd if pi in POOL_PAIRS else nc.vector
                xw = xq[grp * 2 + ci][mslot][:]
                part = list(xw.ap[0])
                win = AP(tensor=xw.tensor, offset=xw.offset + (7 * WP if h else 0),
                         ap=[part, [1, K], [WP, K], [1, W]])
                eng.tensor_mul(
                    PV[:].rearrange("p k (r w) -> p k r w", r=K),
                    ck4[grp][:][:, :, o:o + n].rearrange(
                        "p k (r w) -> p k r w", r=K),
                    win)
                S_ap = smat[:, grp * 4 + mslot]
                if pi in KNOB_PAIRS:
                    PW = pvw.tile([128, 3, 392], bf16, tag="PW", name=f"PW{pi}")
                    nc.vector.tensor_add(PW[:, 0:2], PV[:, 0:2], PV[:, 2:4])
                    nc.vector.tensor_add(PW[:, 2], PV[:, 4], PV[:, 5])
                    nc.vector.tensor_add(PW[:, 2], PW[:, 2], PV[:, 6])
                    planes, src_t = 3, PW
                else:
                    planes, src_t = K, PV
                for kx in range(planes):
                    last = (pi == last_pair[(ci, h)]) and kx == planes - 1
                    nc.tensor.matmul(pre_ps[ci][h][:],
                                     S_ap, src_t[:, kx, :],
                                     start=first[ci][h], stop=last)
                    first[ci][h] = False
                if pi == 0:
                    emit_norm(1, 0)
                if pi == 1:
                    emit_norm(1, 1)

            # ---------------- final 1x1 conv ----------------
            # per split: copy both ci's pre to SBUF, then run the out matmuls
            # for that split while the other split's copies proceed.
            pre_sb = [outp.tile([128, L], bf16, tag=f"psb{ci}", name=f"psb{ci}")
                      for ci in range(2)]
            y_sb = outp.tile([128, 2, L], bf16, tag="ysb", name="ysb")
            ops = [[None] * 2 for _ in range(2)]
            for si, (o, n) in enumerate(SPLITS):
                for ci in range(2):
                    nc.scalar.copy(pre_sb[ci][:][:, o:o + n], pre_ps[ci][si][:])
                for oc in range(2):
                    ps = psB.tile([128, 512], f32, tag="pso", name=f"ps_o{oc}{si}")
                    for ci in range(2):
                        nc.tensor.matmul(ps[:, :n], fw[:, 2 * ci + oc],
                                         pre_sb[ci][:][:, o:o + n],
                                         start=(ci == 0), stop=(ci == 1))
                    ops[oc][si] = ps
                    if si == 0:
                        nc.scalar.activation(y_sb[:, oc, o:o + n], ps[:, :n],
                                             Ident, bias=fb[:, oc:oc + 1],
                                             scale=1.0)
            # split-1 bias-copies go on DVE/Pool so they run concurrently
            o1, n1 = SPLITS[1]
            nc.vector.tensor_scalar_add(y_sb[:, 0, o1:o1 + n1],
                                        ops[0][1][:, :n1], fb[:, 0:1])
            nc.scalar.activation(y_sb[:, 1, o1:o1 + n1], ops[1][1][:, :n1],
                                 Ident, bias=fb[:, 1:2], scale=1.0)
            for oc in range(2):
                nc.sync.dma_start(y_d.ap()[:, oc, 0:SPLITS[0][1]],
                                  y_sb[:][:, oc, 0:SPLITS[0][1]])
            o1, n1 = SPLITS[1]
            for oc in range(2):
                nc.sync.dma_start(y_d.ap()[:, oc, o1:o1 + n1],
                                  y_sb[:][:, oc, o1:o1 + n1])

    nc.compile()
    return nc


def _get_program():
    global _PROGRAM
    if _PROGRAM is None:
        _PROGRAM = _build_program()
    return _PROGRAM


def _gpk_host(gp_w1, gp_b1, gp_w2, gp_b2):
    """GeometryPrior on host (tiny: 49 positions through a 2->16->32 MLP)."""
    a = np.arange(-(K // 2), K // 2 + 1, dtype=np.float32)
    x_pos = np.broadcast_to(a[None, :], (K, K))
    y_pos = np.broadcast_to(a[::-1][:, None], (K, K))
    pos = np.stack([x_pos, y_pos], 0).astype(np.float32)          # [2,7,7]
    h1 = np.einsum('pij,mp->mij', pos, np.asarray(gp_w1, np.float32))
    h1 = np.maximum(h1 + np.asarray(gp_b1, np.float32)[:, None, None], 0.0)
    gpk = np.einsum('mij,cm->cij', h1, np.asarray(gp_w2, np.float32))
    gpk = gpk + np.asarray(gp_b2, np.float32)[:, None, None]      # [32,7,7]
    return gpk


def make_inputs(x, k_w, k_b, q_w, q_b, gp_w1, gp_b1, gp_w2, gp_b2, f_w, f_b):
    """Returns per-core input maps (list of 8 dicts)."""
    x = np.asarray(x, np.float32)
    xp = np.zeros((B, C, HP, WP), np.float32)
    xp[:, :, PAD:PAD + H, PAD:PAD + W] = x

    # channel order: chunk ci partition p -> c = (4ci + p//32)*32 + p%32
    chan = np.arange(128)
    c_of = [((4 * ci + chan // 32) * 32 + chan % 32) for ci in range(2)]

    k_w = np.asarray(k_w, np.float32)
    q_w = np.asarray(q_w, np.float32)
    f_w = np.asarray(f_w, np.float32)

    # wk[p, ci, cm] = k_w[cm, c_of[ci][p]]
    wk = np.stack([k_w[:, c_of[ci]].T for ci in range(2)], 1).astype(_bf16)
    # wq[p, ci, cm*4+g] = q_w[cm, c_of[ci][p]]
    wq_rows = np.stack([q_w[:, c_of[ci]].T for ci in range(2)], 1)  # [128,2,32]
    wq = np.repeat(wq_rows, 4, axis=2)                              # cm*4+g
    wq = wq.reshape(128, 2, CM, 4).reshape(128, 2, 128).astype(_bf16)

    # selection matrices: sm[k, grp*4+mslot, q] = 1 iff k = (q%32)*4+g valid g
    # slot 8 is the identity (for the softmax-denominator sums on PE)
    sm = np.zeros((128, 9, 128), np.float32)
    for grp in range(2):
        ng = 4 if grp == 0 else 3
        for mslot in range(4):
            for cm in range(CM):
                for g in range(ng):
                    sm[cm * 4 + g, grp * 4 + mslot, mslot * 32 + cm] = 1.0
    sm[:, 8, :] = np.eye(128, dtype=np.float32)
    sm = sm.astype(_bf16)

    # fw[p, 2*ci+oc, q] = f_w[c_of[oc][q], c_of[ci][p]]
    fw = np.zeros((128, 4, 128), np.float32)
    for ci in range(2):
        for oc in range(2):
            fw[:, 2 * ci + oc, :] = f_w[np.ix_(c_of[oc], c_of[ci])].T
    fw = fw.astype(_bf16)

    gpk_full = _gpk_host(gp_w1, gp_b1, gp_w2, gp_b2)      # [32,7,7]
    gpk = np.zeros((128, 2 * K), np.float32)
    for cm in range(CM):
        for g in range(4):
            gpk[cm * 4 + g, 0:K] = gpk_full[cm, g]
            gpk[cm * 4 + g, K:2 * K] = gpk_full[cm, 4 + min(g, 2)]

    kb = np.asarray(k_b, np.float32).reshape(CM, 1)
    qb = np.repeat(np.asarray(q_b, np.float32), 4).reshape(128, 1)
    fb = np.stack([np.asarray(f_b, np.float32)[c_of[oc]] for oc in range(2)],
                  1)                                       # [128, 2]

    xp16 = xp.astype(_bf16)
    in_maps = []
    for core in range(NCORES):
        b, rb = divmod(core, RB)
        sl = np.zeros((128, 2, RHP, WP), _bf16)
        r0 = rb * RH
        nrows = min(RHP, HP - r0)
        for ci in range(2):
            sl[:, ci, :nrows] = xp16[b, c_of[ci], r0:r0 + nrows, :]
        in_maps.append({
            "xp": sl, "wk": wk, "wq": wq, "sm": sm, "fw": fw, "gpk": gpk,
            "kb": kb, "qb": qb, "fb": fb,
        })
    return in_maps


def assemble(results):
    chan = np.arange(128)
    out = np.empty((B, C, H, W), np.float32)
    for core in range(NCORES):
        b, rb = divmod(core, RB)
        y = np.asarray(results[core]["y"], np.float32)     # [128, 2, 784]
        for oc in range(2):
            c_of = (4 * oc + chan // 32) * 32 + chan % 32
            out[b, c_of, rb * RH:(rb + 1) * RH, :] = y[:, oc].reshape(
                128, RH, W)
    return out


def kernel(**inputs):
    from concourse import bass_utils
    nc = _get_program()
    in_maps = make_inputs(**inputs)
    res = bass_utils.run_bass_kernel_spmd(nc, in_maps, list(range(NCORES)))
    return assemble(res.results)


# revision 40
# speedup vs baseline: 1.0058x; 1.0058x over previous
"""Trainium2 Bass kernel for LocalRelationalLayer (sparse_attention).

Computation (per reference):
  xp = zero-pad(x, 3)                                   # [B,256,62,62]
  km = 1x1conv(xp, k_w)+k_b ; qm = 1x1conv(xp, q_w)+q_b # [B,32,.,.]
  E[b,cm,l,ky,kx] = exp(km[b,cm,r+ky,w+kx]*qm[b,cm,r+3,w+3] + gpk[cm,ky,kx])
  ck = E / sum_kx E                                     # softmax over kx only
  pre[b,m*32+cm,l] = sum_{ky,kx} ck * xp[b,m*32+cm,r+ky,w+kx]
  out = 1x1conv(pre, f_w)+f_b                           # [B,256,56,56]

Sharding: 8 cores = (b in 2) x (4 row-blocks of 14 output rows); halo rows in
the per-core slice; host concatenates. No collectives.

Per-core strategy ("all-packed"):
  - Attention weights are computed in a PACKED partition layout p = cm*4+g
    where g indexes ky within a group (group A: ky 0-3, group B: ky 4-6,
    slot g=3 duplicated/ignored).  This cuts the exp count 49 -> 14 and the
    softmax arithmetic 3.5x vs computing them 4x-replicated.
  - km is computed once on 32 partitions; cheap DMA remaps build the
    ky-shifted packed views (km4) and the per-head-group packed value views
    (xv_qm) -- one descriptor-friendly DMA each.
  - softmax denominators invert via the single-instruction DVE fast
    reciprocal (no ACT Ln/Exp table thrashing).
  - The value products PV = ck (x) xv happen as ONE fused DVE/Pool
    instruction per (group, head) over [128, 7kx, 784] using overlapping
    window access patterns.
  - All kx / ky-group / head-group summation is done by the otherwise-idle
    TensorEngine: selection matrices S (precomputed 0/1) matmul-accumulate
    every PV plane into the pre[] PSUM accumulators, also undoing the packed
    layout.  The final 1x1 conv then runs from SBUF copies of pre.
"""

import numpy as np
import ml_dtypes

B, C, H, W = 2, 256, 56, 56
K, PAD, M, CM = 7, 3, 8, 32
HP, WP = H + 2 * PAD, W + 2 * PAD      # 62, 62
RB = 4                                  # row blocks per batch
RH = H // RB                            # 14 output rows per core
RHP = RH + K                            # 21 stored rows per core (20 + 1 junk)
NCORES = 8
L = RH * W                              # 784 output positions per core

_bf16 = ml_dtypes.bfloat16
_PROGRAM = None

# column splits of the 784 positions into PSUM-bank-sized pieces
SPLITS = ((0, 392), (392, 392))


def _build_program():
    import concourse.bass as bass
    import concourse.tile as tile
    from concourse import bacc, mybir
    from concourse.ap import AP

    f32 = mybir.dt.float32
    bf16 = mybir.dt.bfloat16
    Exp = mybir.ActivationFunctionType.Exp
    Ident = mybir.ActivationFunctionType.Identity
    PS = bass.MemorySpace.PSUM

    nc = bacc.Bacc("TRN2", target_bir_lowering=False, debug=False,
                   num_devices=NCORES)

    xp_d = nc.dram_tensor("xp", [128, 2, RHP, WP], bf16, kind="ExternalInput")
    wk_d = nc.dram_tensor("wk", [128, 2, CM], bf16, kind="ExternalInput")
    wq_d = nc.dram_tensor("wq", [128, 2, 128], bf16, kind="ExternalInput")
    sm_d = nc.dram_tensor("sm", [128, 9, 128], bf16, kind="ExternalInput")
    fw_d = nc.dram_tensor("fw", [128, 4, 128], bf16, kind="ExternalInput")
    gpk_d = nc.dram_tensor("gpk", [128, 2 * K], f32, kind="ExternalInput")
    kb_d = nc.dram_tensor("kb", [CM, 1], f32, kind="ExternalInput")
    qb_d = nc.dram_tensor("qb", [128, 1], f32, kind="ExternalInput")
    fb_d = nc.dram_tensor("fb", [128, 2], f32, kind="ExternalInput")
    y_d = nc.dram_tensor("y", [128, 2, L], bf16, kind="ExternalOutput")

    NKM = RHP * WP          # 1302
    NQM = RH * WP           # 868

    def win_ap(t4):
        """[128, 7, 14, 56] overlapping kx-window view of a [128,14,62] tile."""
        a = t4[:]
        part = list(a.ap[0])
        return AP(tensor=a.tensor, offset=a.offset,
                  ap=[part, [1, K], [WP, RH], [1, W]])

    with tile.TileContext(nc) as tc:
        with (
            tc.tile_pool(name="inp", bufs=1) as inp,
            tc.tile_pool(name="wpool", bufs=1) as wpool,
            tc.tile_pool(name="kq", bufs=1) as kq,
            tc.tile_pool(name="att", bufs=1) as att,
            tc.tile_pool(name="pv", bufs=7) as pvp,
            tc.tile_pool(name="pvw", bufs=2) as pvw,
            tc.tile_pool(name="outp", bufs=1) as outp,
            tc.tile_pool(name="psMM", bufs=2, space=PS) as psMM,
            tc.tile_pool(name="psA", bufs=1, space=PS) as psA,
            tc.tile_pool(name="psB", bufs=2, space=PS) as psB,
        ):
            # ---- ACT table preload: absorb the 1283ns load off the path ----
            scratch = wpool.tile([32, 1], f32, tag="scr", name="scr")
            nc.gpsimd.memset(scratch[:], 0.0)
            nc.scalar.activation(scratch[:], scratch[:], Exp, bias=0.0,
                                 scale=1.0)

            # ---------------- input DMAs (SP queue, in need-order) -------
            xv = inp.tile([128, 2, RHP, WP], bf16, tag="xv", name="xv")
            nc.sync.dma_start(xv[:][:, 0], xp_d.ap()[:, 0])
            wk = wpool.tile([128, 2, CM], bf16, tag="wk", name="wk")
            nc.sync.dma_start(wk[:], wk_d.ap())
            kb = wpool.tile([CM, 1], f32, tag="kb", name="kb")
            nc.sync.dma_start(kb[:], kb_d.ap())
            nc.sync.dma_start(xv[:][:, 1], xp_d.ap()[:, 1])
            qb = wpool.tile([128, 1], f32, tag="qb", name="qb")
            nc.sync.dma_start(qb[:], qb_d.ap())
            wq = wpool.tile([128, 2, 128], bf16, tag="wq", name="wq")
            nc.sync.dma_start(wq[:], wq_d.ap())
            gpk = wpool.tile([128, 2 * K], f32, tag="gpk", name="gpk")
            nc.sync.dma_start(gpk[:], gpk_d.ap())
            smat = wpool.tile([128, 9, 128], bf16, tag="sm", name="sm")
            nc.sync.dma_start(smat[:], sm_d.ap())

            xq = [[None] * 4 for _ in range(4)]  # [grp*2+ci][mslot]

            def emit_xq(grp, ci, mslot):
                base = 0 if grp == 0 else 4
                t = kq.tile([128, RH, WP], bf16,
                            tag=f"xq{grp}{ci}{mslot}",
                            name=f"xq{grp}{ci}{mslot}")
                src0 = xv[:][mslot * 32:(mslot + 1) * 32, ci]
                part = list(src0.ap[0])
                src = AP(tensor=src0.tensor,
                         offset=src0.offset + base * WP,
                         ap=[part, [WP, 4], [WP, RH], [1, WP]])
                nc.sync.dma_start(t[:], src)
                xq[grp * 2 + ci][mslot] = t

            emit_xq(0, 0, 0)
            emit_xq(0, 0, 1)

            # km4 remaps are on the SP queue *early*: they gate P4/exp
            km32 = kq.tile([CM, RHP, WP], bf16, tag="km32", name="km32")
            km4 = []
            for grp in range(2):
                km4.append(kq.tile([128, RH, WP], bf16, tag=f"km4{grp}",
                                   name=f"km4{grp}"))

            def emit_km4(grp, h):
                # half h covers km4 rows 7h..7h+6  (km32 rows base+g+7h..)
                base = (0 if grp == 0 else 4) + 7 * h
                a = km32[:]
                part = list(a.ap[0])
                src = AP(tensor=a.tensor, offset=a.offset + base * WP,
                         ap=[part, [WP, 4], [WP, K], [1, WP]])
                nc.sync.dma_start(km4[grp][:][:, 7 * h:7 * h + K, :], src)

            # ---------------- km32 / qm4 matmuls ----------------
            km32_f = km32[:].rearrange("p r w -> p (r w)")
            for si, off in enumerate(range(0, NKM, 512)):
                n = min(512, NKM - off)
                ps = psMM.tile([128, 512], f32, tag="mm", name=f"psk{off}")
                for ci in range(2):
                    rhs = xv[:].rearrange("p c r w -> p (c r w)")[
                        :, ci * NKM + off: ci * NKM + off + n]
                    nc.tensor.matmul(ps[:CM, :n], wk[:, ci], rhs,
                                     start=(ci == 0), stop=(ci == 1))
                if si == 0:
                    nc.scalar.activation(km32_f[:, off:off + n], ps[:CM, :n],
                                         Ident, bias=kb[:], scale=1.0)
                elif si == 1:
                    nc.vector.tensor_scalar_add(km32_f[:, off:off + n],
                                                ps[:CM, :n], kb[:])
                else:
                    nc.vector.tensor_scalar_add(km32_f[:, off:off + n],
                                                ps[:CM, :n], kb[:])
            emit_km4(0, 0)
            emit_km4(0, 1)
            emit_km4(1, 0)
            emit_km4(1, 1)
            qm4 = kq.tile([128, RH, WP], bf16, tag="qm4", name="qm4")
            qm4_f = qm4[:].rearrange("p r w -> p (r w)")
            for off in range(0, NQM, 512):
                n = min(512, NQM - off)
                ps = psMM.tile([128, 512], f32, tag="mm", name=f"psq{off}")
                for ci in range(2):
                    rhs = xv[:].rearrange("p c r w -> p (c r w)")[
                        :, ci * NKM + PAD * WP + off: ci * NKM + PAD * WP + off + n]
                    nc.tensor.matmul(ps[:, :n], wq[:, ci], rhs,
                                     start=(ci == 0), stop=(ci == 1))
                if off == 0:
                    nc.scalar.activation(qm4_f[:, off:off + n], ps[:, :n],
                                         Ident, bias=qb[:], scale=1.0)
                else:
                    nc.vector.tensor_scalar_add(qm4_f[:, off:off + n],
                                                ps[:, :n], qb[:])

            # remaining value-view remaps + late weights on SP
            emit_xq(0, 0, 2)
            emit_xq(0, 0, 3)
            for mslot in range(4):
                emit_xq(0, 1, mslot)
            for ci in range(2):
                for mslot in range(4):
                    emit_xq(1, ci, mslot)
            fw = wpool.tile([128, 4, 128], bf16, tag="fw", name="fw")
            nc.sync.dma_start(fw[:], fw_d.ap())
            fb = wpool.tile([128, 2], f32, tag="fb", name="fb")
            nc.sync.dma_start(fb[:], fb_d.ap())

            qmc = qm4[:][:, :, PAD:PAD + W]  # [128, 14, 56] center query
            ident = smat[:, 8]               # [128, 128] identity

            def bcast_kx(ap2d):
                return ap2d.unsqueeze(1).broadcast_to((128, K, ap2d.shape[1]))

            # ---------------- attention (packed) ----------------
            # per-kx P4 muls; exps/d/recip/ck all split into position halves
            # (rows 0:7 / 7:14) so the normalized weights stream out early.
            P4 = [att.tile([128, K, RH, W], bf16, tag=f"P4{g}", name=f"P4{g}")
                  for g in range(2)]
            E4 = [att.tile([128, K, L], bf16, tag=f"E4{g}", name=f"E4{g}")
                  for g in range(2)]
            dps = [[psB.tile([128, n], f32, tag="pso", name=f"d{g}{si}")
                    for si, (o, n) in enumerate(SPLITS)] for g in range(2)]
            rf = [att.tile([128, L], f32, tag=f"rf{g}", name=f"rf{g}")
                  for g in range(2)]
            rb = [att.tile([128, L], bf16, tag=f"rb{g}", name=f"rb{g}")
                  for g in range(2)]
            ck4 = [att.tile([128, K, L], bf16, tag=f"ck{g}", name=f"ck{g}")
                   for g in range(2)]

            def emit_attention(grp):
                nsplit = 3 if grp == 0 else 2
                for h in range(2):
                    for kx in range(K):
                        eng = nc.vector if kx < nsplit else nc.gpsimd
                        eng.tensor_mul(
                            P4[grp][:, kx, 7 * h:7 * h + K, :],
                            km4[grp][:][:, 7 * h:7 * h + K, kx:kx + W],
                            qmc[:, 7 * h:7 * h + K, :])
                for h, (o, n) in enumerate(SPLITS):
                    for kx in range(K):
                        nc.scalar.activation(
                            E4[grp][:, kx, o:o + n],
                            P4[grp][:, kx].rearrange(
                                "p r w -> p (r w)")[:, o:o + n], Exp,
                            bias=gpk[:, grp * K + kx:grp * K + kx + 1],
                            scale=1.0)

            df_sb = att.tile([128, L], f32, tag="df0", name="df0")

            def emit_dsum(grp):
                for si, (o, n) in enumerate(SPLITS):
                    for kx in range(K):
                        nc.tensor.matmul(dps[grp][si][:], ident,
                                         E4[grp][:, kx, o:o + n],
                                         start=(kx == 0), stop=(kx == K - 1))

            def emit_dsum_pool(grp):
                # pairwise tree on the (idle) Pool; df_sb in f32 for the recip
                for h, (o, n) in enumerate(SPLITS):
                    E = E4[grp][:]
                    a = att.tile([128, 2, 392], bf16, tag=f"da{h}", name=f"da{grp}{h}")
                    nc.gpsimd.tensor_add(a[:], E[:, 0:2, o:o + n], E[:, 2:4, o:o + n])
                    b = att.tile([128, 392], bf16, tag=f"db{h}", name=f"db{grp}{h}")
                    nc.gpsimd.tensor_add(b[:], E[:, 4, o:o + n], E[:, 5, o:o + n])
                    c = att.tile([128, 392], bf16, tag=f"dc{h}", name=f"dc{grp}{h}")
                    nc.gpsimd.tensor_add(c[:], a[:, 0], a[:, 1])
                    d = att.tile([128, 392], bf16, tag=f"dd{h}", name=f"dd{grp}{h}")
                    nc.gpsimd.tensor_add(d[:], c[:], b[:])
                    nc.gpsimd.tensor_add(df_sb[:][:, o:o + n], d[:],
                                         E[:, 6, o:o + n])

            def emit_norm(grp, h):
                from concourse.dve_ops import (RECIPROCAL_APPROX_FAST,
                                               RECIP_APPROX_FAST_CONSTS)
                o, n = SPLITS[h]
                dsrc = dps[grp][h][:]
                nc.vector._custom_dve(RECIPROCAL_APPROX_FAST,
                                      out=rb[grp][:][:, o:o + n], in0=dsrc,
                                      **RECIP_APPROX_FAST_CONSTS)
                nc.vector.tensor_mul(
                    ck4[grp][:][:, :, o:o + n], E4[grp][:][:, :, o:o + n],
                    rb[grp][:][:, o:o + n].unsqueeze(1).broadcast_to(
                        (128, K, n)))

            emit_attention(0)
            emit_dsum(0)
            emit_attention(1)
            emit_norm(0, 0)
            emit_dsum(1)
            emit_norm(0, 1)

            # ---------------- value phase ----------------
            pre_ps = [[psA.tile([128, n], f32, tag=f"pre{ci}{si}",
                                name=f"pre{ci}{si}")
                       for si, (o, n) in enumerate(SPLITS)] for ci in range(2)]
            first = [[True] * 2 for _ in range(2)]
            UNITS = [(grp, ci, mslot) for grp in range(2)
                     for ci in range(2) for mslot in range(4)]
            # (ui, h) pairs: all grp-A half-0, then grp-A half-1, then grp-B
            PAIRS = ([(ui, 0) for ui in range(8)] + [(ui, 1) for ui in range(8)]
                     + [(ui, 0) for ui in range(8, 16)]
                     + [(ui, 1) for ui in range(8, 16)])
            last_pair = {}
            for pi, (ui, h) in enumerate(PAIRS):
                last_pair[(UNITS[ui][1], h)] = pi
            POOL_PAIRS = set(range(0, 32, 2))  # alternate Pool/DVE halves
            KNOB_PAIRS = {24, 26, 28, 30}      # Pool-produced, DVE presums
            for pi, (ui, h) in enumerate(PAIRS):
                grp, ci, mslot = UNITS[ui]
                o, n = SPLITS[h]
                PV = pvp.tile([128, K, 392], bf16, tag="PV", name=f"PV{pi}")
                eng = nc.gpsimd if pi in POOL_PAIRS else nc.vector
                xw = xq[grp * 2 + ci][mslot][:]
                part = list(xw.ap[0])
                win = AP(tensor=xw.tensor, offset=xw.offset + (7 * WP if h else 0),
                         ap=[part, [1, K], [WP, K], [1, W]])
                eng.tensor_mul(
                    PV[:].rearrange("p k (r w) -> p k r w", r=K),
                    ck4[grp][:][:, :, o:o + n].rearrange(
                        "p k (r w) -> p k r w", r=K),
                    win)
                S_ap = smat[:, grp * 4 + mslot]
                if pi in KNOB_PAIRS:
                    PW = pvw.tile([128, 3, 392], bf16, tag="PW", name=f"PW{pi}")
                    nc.vector.tensor_add(PW[:, 0:2], PV[:, 0:2], PV[:, 2:4])
                    nc.vector.tensor_add(PW[:, 2], PV[:, 4], PV[:, 5])
                    nc.vector.tensor_add(PW[:, 2], PW[:, 2], PV[:, 6])
                    planes, src_t = 3, PW
                else:
                    planes, src_t = K, PV
                for kx in range(planes):
                    last = (pi == last_pair[(ci, h)]) and kx == planes - 1
                    nc.tensor.matmul(pre_ps[ci][h][:],
                                     S_ap, src_t[:, kx, :],
                                     start=first[ci][h], stop=last)
                    first[ci][h] = False
                if pi == 0:
                    emit_norm(1, 0)
                if pi == 1:
                    emit_norm(1, 1)

            # ---------------- final 1x1 conv ----------------
            # per split: copy both ci's pre to SBUF, then run the out matmuls
            # for that split while the other split's copies proceed.
            pre_sb = [outp.tile([128, L], bf16, tag=f"psb{ci}", name=f"psb{ci}")
                      for ci in range(2)]
            y_sb = outp.tile([128, 2, L], bf16, tag="ysb", name="ysb")
            ops = [[None] * 2 for _ in range(2)]
            for si, (o, n) in enumerate(SPLITS):
                for ci in range(2):
                    if si == 1 and ci == 1:
                        nc.vector.tensor_scalar_mul(
                            pre_sb[ci][:][:, o:o + n], pre_ps[ci][si][:], 1.0)
                    else:
                        nc.scalar.copy(pre_sb[ci][:][:, o:o + n],
                                       pre_ps[ci][si][:])
                for oc in range(2):
                    ps = psB.tile([128, 512], f32, tag="pso", name=f"ps_o{oc}{si}")
                    for ci in range(2):
                        nc.tensor.matmul(ps[:, :n], fw[:, 2 * ci + oc],
                                         pre_sb[ci][:][:, o:o + n],
                                         start=(ci == 0), stop=(ci == 1))
                    ops[oc][si] = ps
                    if si == 0:
                        nc.scalar.activation(y_sb[:, oc, o:o + n], ps[:, :n],
                                             Ident, bias=fb[:, oc:oc + 1],
                                             scale=1.0)
            # split-1 bias-copies go on DVE/Pool so they run concurrently
            o1, n1 = SPLITS[1]
            nc.vector.tensor_scalar_add(y_sb[:, 0, o1:o1 + n1],
                                        ops[0][1][:, :n1], fb[:, 0:1])
            nc.scalar.activation(y_sb[:, 1, o1:o1 + n1], ops[1][1][:, :n1],
                                 Ident, bias=fb[:, 1:2], scale=1.0)
            nc.sync.dma_start(y_d.ap()[:, 0, 0:SPLITS[0][1]],
                              y_sb[:][:, 0, 0:SPLITS[0][1]])
            nc.scalar.dma_start(y_d.ap()[:, 1, 0:SPLITS[0][1]],
                                y_sb[:][:, 1, 0:SPLITS[0][1]])
            o1, n1 = SPLITS[1]
            nc.sync.dma_start(y_d.ap()[:, 0, o1:o1 + n1],
                              y_sb[:][:, 0, o1:o1 + n1])
            nc.scalar.dma_start(y_d.ap()[:, 1, o1:o1 + n1],
                                y_sb[:][:, 1, o1:o1 + n1])

    nc.compile()
    return nc


def _get_program():
    global _PROGRAM
    if _PROGRAM is None:
        _PROGRAM = _build_program()
    return _PROGRAM


def _gpk_host(gp_w1, gp_b1, gp_w2, gp_b2):
    """GeometryPrior on host (tiny: 49 positions through a 2->16->32 MLP)."""
    a = np.arange(-(K // 2), K // 2 + 1, dtype=np.float32)
    x_pos = np.broadcast_to(a[None, :], (K, K))
    y_pos = np.broadcast_to(a[::-1][:, None], (K, K))
    pos = np.stack([x_pos, y_pos], 0).astype(np.float32)          # [2,7,7]
    h1 = np.einsum('pij,mp->mij', pos, np.asarray(gp_w1, np.float32))
    h1 = np.maximum(h1 + np.asarray(gp_b1, np.float32)[:, None, None], 0.0)
    gpk = np.einsum('mij,cm->cij', h1, np.asarray(gp_w2, np.float32))
    gpk = gpk + np.asarray(gp_b2, np.float32)[:, None, None]      # [32,7,7]
    return gpk


def make_inputs(x, k_w, k_b, q_w, q_b, gp_w1, gp_b1, gp_w2, gp_b2, f_w, f_b):
    """Returns per-core input maps (list of 8 dicts)."""
    x = np.asarray(x, np.float32)
    xp = np.zeros((B, C, HP, WP), np.float32)
    xp[:, :, PAD:PAD + H, PAD:PAD + W] = x

    # channel order: chunk ci partition p -> c = (4ci + p//32)*32 + p%32
    chan = np.arange(128)
    c_of = [((4 * ci + chan // 32) * 32 + chan % 32) for ci in range(2)]

    k_w = np.asarray(k_w, np.float32)
    q_w = np.asarray(q_w, np.float32)
    f_w = np.asarray(f_w, np.float32)

    # wk[p, ci, cm] = k_w[cm, c_of[ci][p]]
    wk = np.stack([k_w[:, c_of[ci]].T for ci in range(2)], 1).astype(_bf16)
    # wq[p, ci, cm*4+g] = q_w[cm, c_of[ci][p]]
    wq_rows = np.stack([q_w[:, c_of[ci]].T for ci in range(2)], 1)  # [128,2,32]
    wq = np.repeat(wq_rows, 4, axis=2)                              # cm*4+g
    wq = wq.reshape(128, 2, CM, 4).reshape(128, 2, 128).astype(_bf16)

    # selection matrices: sm[k, grp*4+mslot, q] = 1 iff k = (q%32)*4+g valid g
    # slot 8 is the identity (for the softmax-denominator sums on PE)
    sm = np.zeros((128, 9, 128), np.float32)
    for grp in range(2):
        ng = 4 if grp == 0 else 3
        for mslot in range(4):
            for cm in range(CM):
                for g in range(ng):
                    sm[cm * 4 + g, grp * 4 + mslot, mslot * 32 + cm] = 1.0
    sm[:, 8, :] = np.eye(128, dtype=np.float32)
    sm = sm.astype(_bf16)

    # fw[p, 2*ci+oc, q] = f_w[c_of[oc][q], c_of[ci][p]]
    fw = np.zeros((128, 4, 128), np.float32)
    for ci in range(2):
        for oc in range(2):
            fw[:, 2 * ci + oc, :] = f_w[np.ix_(c_of[oc], c_of[ci])].T
    fw = fw.astype(_bf16)

    gpk_full = _gpk_host(gp_w1, gp_b1, gp_w2, gp_b2)      # [32,7,7]
    gpk = np.zeros((128, 2 * K), np.float32)
    for cm in range(CM):
        for g in range(4):
            gpk[cm * 4 + g, 0:K] = gpk_full[cm, g]
            gpk[cm * 4 + g, K:2 * K] = gpk_full[cm, 4 + min(g, 2)]

    kb = np.asarray(k_b, np.float32).reshape(CM, 1)
    qb = np.repeat(np.asarray(q_b, np.float32), 4).reshape(128, 1)
    fb = np.stack([np.asarray(f_b, np.float32)[c_of[oc]] for oc in range(2)],
                  1)                                       # [128, 2]

    xp16 = xp.astype(_bf16)
    in_maps = []
    for core in range(NCORES):
        b, rb = divmod(core, RB)
        sl = np.zeros((128, 2, RHP, WP), _bf16)
        r0 = rb * RH
        nrows = min(RHP, HP - r0)
        for ci in range(2):
            sl[:, ci, :nrows] = xp16[b, c_of[ci], r0:r0 + nrows, :]
        in_maps.append({
            "xp": sl, "wk": wk, "wq": wq, "sm": sm, "fw": fw, "gpk": gpk,
            "kb": kb, "qb": qb, "fb": fb,
        })
    return in_maps


def assemble(results):
    chan = np.arange(128)
    out = np.empty((B, C, H, W), np.float32)
    for core in range(NCORES):
        b, rb = divmod(core, RB)
        y = np.asarray(results[core]["y"], np.float32)     # [128, 2, 784]
        for oc in range(2):
            c_of = (4 * oc + chan // 32) * 32 + chan % 32
            out[b, c_of, rb * RH:(rb + 1) * RH, :] = y[:, oc].reshape(
                128, RH, W)
    return out


def kernel(**inputs):
    from concourse import bass_utils
    nc = _get_program()
    in_maps = make_inputs(**inputs)
    res = bass_utils.run_bass_kernel_spmd(nc, in_maps, list(range(NCORES)))
    return assemble(res.results)
